# revision 21
# baseline (speedup 1.0000x reference)
"""3D Haar DWT low-pass (DWT3DTiny) Trainium2 kernel.

The reference applies the Haar rec_lo filter [s, s] (s = sqrt(2)/2) with
stride-2 downsampling along t, h, w for every channel.  That is exactly a
2x2x2 box sum scaled by s^3 = 2**-1.5:

    out[ts, hs, ws, c] = 2**-1.5 * sum_{dt,dh,dw in {0,1}} x[2ts+dt, 2hs+dh, 2ws+dw, c]

Sharding: along t (pure data-parallel, t-pairs never cross a core
boundary since 32 / 8 = 4 rows per core), contiguous host-side slices.

v8 design, from NTFF packet/instruction analysis.
DMA side: the 16 SDMA engines are ~97% occupied in-span and per-packet
throughput grows with descriptor size (2 KiB -> 24.4, 8 KiB -> 26.0,
32 KiB -> 26.6 GB/s/engine), so the three bulk chunks keep partition p
holding h rows (2p, 2p+1) full width = one 32 KiB contiguous
descriptor per partition per t-row tile.
End side: the run's end is bounded by the serial DVE chain over
last-landing data.  A t-row that completes a pair carries ~2.6 us/MiB
of DVE (hb, t, w) against a ~2.5 us/MiB DMA stream, so a b-only
stream tail can never catch up once the previous chunk's coda
(~10.7 us) is in front of it.  The final chunk is therefore loaded as
interleaved (a, b) w-slice pairs [128,128,128,64,32,16,16] wi, whose
amortized DVE density (~2.1 us/MiB) leaves slack to drain the coda;
one early slice is processed end-to-end on the otherwise idle GpSimd
(Pool) engine as a second compute lane (it is 2.3x slower per
element, so it only gets a slice that lands early); the last two
slices fold the 2**-1.5 scale into the adds (tensor_scalar pre-scale
+ scalar_tensor_tensor t-add) so the post-last-load drain is just a
16-wi chain and one small store.
Loads ride the SP HWDGE ring, stores the ACT ring; the dead
const-tile memsets are stripped from the init preamble (~9 us of
GpSimd startup the all-engine barrier otherwise waits on).
Rejected experimentally: all w-adds on Pool (sem latency on every
store path: 97.2 us), b-rows halved everywhere (desc penalty: 93.4),
b-only graduated tail pieces (92.0-93.4: codas stack), uniform small
tail pieces (92.2), small-desc slices with bufs=2 ring coupling
(95.6), SWDGE loads, 4 MiB loads with bufs=2 (v1 notes).
"""

import numpy as np

import concourse.bacc as bacc
import concourse.mybir as mybir
from concourse.bass_utils import run_bass_kernel_spmd
from concourse.tile import TileContext

N_CORES = 8
T, H, W, C = 32, 512, 512, 8
TS = T // N_CORES  # t rows per core
SCALE = float(2.0 ** -1.5)
WC = W * C  # 4096 f32 per h row
SLICE_WI = [128, 128, 128, 48, 32, 32, 16]  # final chunk (a,b) slice pairs
POOL_SLICE = 1  # this slice runs on GpSimd end-to-end (lands early)
FOLD_FROM = 5  # slices >= this fold the scale (no ACT mul on the drain)
N_RING = 3  # the 128-wi slices share ring tags (bufs=2; s2 reuses s0's
# slot only after s0's w-add has consumed it, which happens earlier)

_CACHE: dict = {}


def _build_nc() -> bacc.Bacc:
    nc = bacc.Bacc("TRN2", target_bir_lowering=False)
    x = nc.dram_tensor("x", [TS, H, W, C], mybir.dt.float32, kind="ExternalInput")
    y = nc.dram_tensor(
        "y", [TS // 2, H // 2, W // 2, C], mybir.dt.float32, kind="ExternalOutput"
    )

    # h = gb*256 + p*2 + two; rows 2p, 2p+1 full-width are adjacent in HBM.
    xq = x.rearrange("t (gb p two) w c -> t gb p two (w c)", p=128, two=2)
    # output row g = gb*128 + p: 256 v * 8 c = 8 KiB contiguous per partition
    yq = y.rearrange("s (gb p) w c -> s gb p (w c)", p=128)

    add = mybir.AluOpType.add
    mult = mybir.AluOpType.mult

    chunks = [(tp, gb) for tp in range(TS // 2) for gb in range(H // 256)]
    LAST = len(chunks) - 1
    ttp, tgb = chunks[LAST]

    def wadd(eng, src, wdst):
        # w-pair add (wi = v*2 + dw): src [128, n] -> wdst [128, n//2]
        hv = src.rearrange("p (v two c) -> p v two c", two=2, c=C)
        wv = wdst.rearrange("p (v c) -> p v c", c=C)
        eng.tensor_add(out=wv[:], in0=hv[:, :, 0], in1=hv[:, :, 1])

    with TileContext(nc) as tc:
        with (
            tc.tile_pool(name="pin", bufs=2) as pin,
            tc.tile_pool(name="pw", bufs=2) as pw,
            tc.tile_pool(name="psl", bufs=2) as ps,
            tc.tile_pool(name="ptl", bufs=1) as pt,
        ):
            # --- bulk chunks (v2-proven structure) -----------------------
            for tp, gb in chunks[:-1]:
                a = pin.tile([128, 2, WC], mybir.dt.float32, tag="a")
                b = pin.tile([128, 2, WC], mybir.dt.float32, tag="b")
                nc.sync.dma_start(out=a[:], in_=xq[2 * tp, gb])
                nc.sync.dma_start(out=b[:], in_=xq[2 * tp + 1, gb])
                nc.vector.tensor_add(out=a[:, 0], in0=a[:, 0], in1=a[:, 1])
                nc.vector.tensor_add(out=b[:, 0], in0=b[:, 0], in1=b[:, 1])
                nc.vector.tensor_add(out=a[:, 0], in0=a[:, 0], in1=b[:, 0])
                ws = pw.tile([128, WC // 2], mybir.dt.float32, tag="w")
                wadd(nc.vector, a[:, 0], ws[:])
                nc.scalar.mul(ws[:], ws[:], SCALE)
                nc.scalar.dma_start(out=yq[tp, gb], in_=ws[:])

            # --- final chunk: interleaved (a, b) w-slice pairs -----------
            w0 = 0
            for k, wi in enumerate(SLICE_WI):
                wc = wi * C
                if k < N_RING:
                    sa = ps.tile([128, 2, wc], mybir.dt.float32, tag="sa")
                    sb = ps.tile([128, 2, wc], mybir.dt.float32, tag="sb")
                    wfull = pw.tile([128, WC // 2], mybir.dt.float32, tag="w")
                    wt = wfull[:, : wc // 2]
                else:
                    sa = pt.tile([128, 2, wc], mybir.dt.float32, tag=f"sa{k}")
                    sb = pt.tile([128, 2, wc], mybir.dt.float32, tag=f"sb{k}")
                    wt = pt.tile([128, wc // 2], mybir.dt.float32, tag=f"sw{k}")
                wsl, wsh = w0 * C, (w0 + wi) * C
                nc.sync.dma_start(out=sa[:], in_=xq[2 * ttp, tgb, :, :, wsl:wsh])
                nc.sync.dma_start(out=sb[:], in_=xq[2 * ttp + 1, tgb, :, :, wsl:wsh])
                eng = nc.gpsimd if k == POOL_SLICE else nc.vector
                fold = k >= FOLD_FROM
                # ha always on DVE (cheap, runs while b streams in)
                nc.vector.tensor_add(out=sa[:, 0], in0=sa[:, 0], in1=sa[:, 1])
                if fold:
                    nc.vector.tensor_scalar_mul(sa[:, 0], sa[:, 0], SCALE)
                eng.tensor_add(out=sb[:, 0], in0=sb[:, 0], in1=sb[:, 1])
                if fold:
                    # t-add with the scale folded: s*hb + (s*ha) -> final
                    nc.vector.scalar_tensor_tensor(
                        out=sb[:, 0], in0=sb[:, 0], scalar=SCALE,
                        in1=sa[:, 0], op0=mult, op1=add,
                    )
                else:
                    eng.tensor_add(out=sb[:, 0], in0=sb[:, 0], in1=sa[:, 0])
                wadd(nc.vector if fold else eng, sb[:, 0], wt[:])
                if not fold:
                    nc.scalar.mul(wt[:], wt[:], SCALE)
                nc.scalar.dma_start(
                    out=yq[ttp, tgb, :, (w0 // 2) * C : ((w0 + wi) // 2) * C],
                    in_=wt[:],
                )
                w0 += wi

    _strip_init_preamble(nc)
    if not nc.is_finalized():
        nc.finalize()  # Bacc.compile: event-sem split (1 wait/inst), reg alloc
    return nc


def _strip_init_preamble(nc) -> None:
    """Drop the four Bass.__init__ const-tile memsets from block 0.  Nothing
    in this kernel reads the const tiles, yet the initial all-engine barrier
    waits on the GpSimd engine executing them, which costs ~9 us of Q7
    startup on HW.  The drains and the all-engine barrier are kept."""
    b0 = nc.main_func.blocks[0]
    b0.instructions[:] = [
        ins for ins in b0.instructions if type(ins).__name__ != "InstMemset"
    ]


def kernel(x) -> np.ndarray:
    x = np.asarray(x, dtype=np.float32)
    assert x.shape == (T, H, W, C), x.shape

    if "nc" not in _CACHE:
        _CACHE["nc"] = _build_nc()
    nc = _CACHE["nc"]

    in_maps = [
        {"x": np.ascontiguousarray(x[i * TS : (i + 1) * TS])} for i in range(N_CORES)
    ]
    res = run_bass_kernel_spmd(nc, in_maps, core_ids=list(range(N_CORES)))
    return np.concatenate([r["y"] for r in res.results], axis=0)
